# revision 2
# baseline (speedup 1.0000x reference)
import sys

sys.path.insert(0, "/opt/trn_rl_repo")

import numpy as np
import ml_dtypes

BF16 = ml_dtypes.bfloat16

B, C, T = 256, 128, 256
H = 256
BS = 64
NCORE = 8
F32 = np.float32

_CACHE = {}


def _build_nc():
    import concourse.bacc as bacc
    import concourse.tile as tile
    from concourse import mybir

    f32 = mybir.dt.float32
    bf16 = mybir.dt.bfloat16
    Sig = mybir.ActivationFunctionType.Sigmoid
    Tanh = mybir.ActivationFunctionType.Tanh

    nc = bacc.Bacc("TRN2", target_bir_lowering=False, debug=False,
                   num_devices=NCORE)

    xq_d = nc.dram_tensor("xq", [4, 128, 4096], bf16, kind="ExternalInput").ap()
    wih_d = nc.dram_tensor("wih", [128, 1024], bf16, kind="ExternalInput").ap()
    whh_d = nc.dram_tensor("whh", [128, 2048], bf16, kind="ExternalInput").ap()
    bias_d = nc.dram_tensor("bias", [8, 128], bf16, kind="ExternalInput").ap()
    oneh_d = nc.dram_tensor("oneh", [8, 512], bf16, kind="ExternalInput").ap()
    hout_d = nc.dram_tensor("hout", [32, 128, 1024], f32,
                            kind="ExternalOutput").ap()

    with tile.TileContext(nc) as tc:
        import contextlib
        with contextlib.ExitStack() as ctx:
            const = ctx.enter_context(tc.tile_pool(name="const", bufs=1))
            psum = ctx.enter_context(
                tc.tile_pool(name="psum", bufs=8, space="PSUM"))
            actp = ctx.enter_context(tc.tile_pool(name="actp", bufs=3))
            dvep = ctx.enter_context(tc.tile_pool(name="dvep", bufs=3))
            statep = ctx.enter_context(tc.tile_pool(name="statep", bufs=1))
            hrowp = ctx.enter_context(tc.tile_pool(name="hrowp", bufs=2))

            wih_sb = const.tile([128, 1024], bf16)
            whh_sb = const.tile([128, 2048], bf16)
            bias_sb = const.tile([8, 128], bf16)
            oneh_sb = const.tile([8, 512], bf16)
            nc.sync.dma_start(wih_sb[:], wih_d[:])
            nc.sync.dma_start(whh_sb[:], whh_d[:])
            nc.sync.dma_start(bias_sb[:], bias_d[:])
            nc.sync.dma_start(oneh_sb[:], oneh_d[:])
            xq_sb = []
            for q in range(4):
                t_ = const.tile([128, 4096], bf16, name=f"xq_sb{q}")
                nc.sync.dma_start(t_[:], xq_d[q])
                xq_sb.append(t_)

            hbf = [statep.tile([128, 128], bf16, name=f"hbf{p}")
                   for p in range(2)]

            psums = {}

            def x_phase(t):
                ps = psum.tile([128, 512], f32, tag="gates", name=f"ps{t}")
                psums[t] = ps
                nc.tensor.matmul(ps[:, :], bias_sb[:, :], oneh_sb[:, :],
                                 start=True, stop=False)
                q, tl = t // 64, t % 64
                for m in range(8):
                    nc.tensor.matmul(
                        ps[:, m * 64:(m + 1) * 64],
                        wih_sb[:, m * 128:(m + 1) * 128],
                        xq_sb[q][:, tl * 64:(tl + 1) * 64],
                        start=False, stop=(t == 0 and m == 7))

            def h_phase(t):
                ps = psums[t]
                hprev = hbf[(t - 1) % 2]
                for mi, m in enumerate((6, 7, 0, 1, 2, 3, 4, 5)):
                    for hc in range(2):
                        nc.tensor.matmul(
                            ps[:, m * 64:(m + 1) * 64],
                            whh_sb[:, hc * 1024 + m * 128:
                                   hc * 1024 + (m + 1) * 128],
                            hprev[:, hc * 64:(hc + 1) * 64],
                            start=False, stop=(mi == 7 and hc == 1))

            c_prev = None
            hrow = None

            def chain(t):
                nonlocal c_prev, hrow
                ps = psums.pop(t)
                tg8 = t % 8
                if tg8 == 0:
                    hrow = hrowp.tile([128, 1024], f32, tag="hrow",
                                      name=f"hrow{t}")
                tgs = actp.tile([128, 128], f32, tag="tg", name=f"tg{t}")
                nc.scalar.activation(tgs[:], ps[:, 384:512], Tanh)
                sif = actp.tile([128, 256], f32, tag="sif", name=f"sif{t}")
                nc.scalar.activation(sif[:], ps[:, 0:256], Sig)
                so = actp.tile([128, 128], f32, tag="so", name=f"so{t}")
                nc.scalar.activation(so[:], ps[:, 256:384], Sig)
                v = dvep.tile([128, 128], f32, tag="v", name=f"v{t}")
                nc.vector.tensor_mul(v[:], sif[:, 0:128], tgs[:])
                if t == 0:
                    c_new = v
                else:
                    cf = dvep.tile([128, 128], f32, tag="cf", name=f"cf{t}")
                    nc.vector.tensor_mul(cf[:], sif[:, 128:256], c_prev[:])
                    c_new = dvep.tile([128, 128], f32, tag="c", name=f"c{t}")
                    nc.vector.tensor_add(c_new[:], cf[:], v[:])
                c_prev = c_new
                tcs = actp.tile([128, 128], f32, tag="tc", name=f"tc{t}")
                nc.scalar.activation(tcs[:], c_new[:], Tanh)
                nc.vector.tensor_mul(hbf[t % 2][:], so[:], tcs[:])
                nc.vector.tensor_mul(
                    hrow[:, tg8 * 128:(tg8 + 1) * 128], so[:], tcs[:])
                if tg8 == 7:
                    nc.sync.dma_start(hout_d[t // 8], hrow[:])

            x_phase(0)
            x_phase(1)
            for t in range(T):
                if t + 2 < T:
                    x_phase(t + 2)
                if t > 0:
                    h_phase(t)
                chain(t)

    nc.finalize()
    return nc


def _prep_dir(W_ih, W_hh, b_ih, b_hh):
    perm = np.concatenate([np.arange(0, 512), np.arange(768, 1024),
                           np.arange(512, 768)])
    wih = np.ascontiguousarray(W_ih[perm, :].T).astype(BF16)
    whh_p = W_hh[perm, :]
    whh = np.concatenate([whh_p[:, 0:128].T, whh_p[:, 128:256].T],
                         axis=1).astype(BF16)
    bias = (b_ih + b_hh)[perm].reshape(8, 128).astype(BF16)
    return {"wih": wih, "whh": whh, "bias": bias}


def _prep_x(x_shard, reverse):
    xt = np.transpose(x_shard, (2, 1, 0))
    if reverse:
        xt = xt[::-1]
    xq = np.ascontiguousarray(
        xt.reshape(4, 64, 128, 64).transpose(0, 2, 1, 3)).reshape(4, 128, 4096)
    return xq.astype(BF16)


def make_in_maps(x, W_ih_f, W_hh_f, b_ih_f, b_hh_f,
                 W_ih_b, W_hh_b, b_ih_b, b_hh_b):
    x = np.asarray(x, dtype=F32)
    wf = _prep_dir(np.asarray(W_ih_f, F32), np.asarray(W_hh_f, F32),
                   np.asarray(b_ih_f, F32), np.asarray(b_hh_f, F32))
    wb = _prep_dir(np.asarray(W_ih_b, F32), np.asarray(W_hh_b, F32),
                   np.asarray(b_ih_b, F32), np.asarray(b_hh_b, F32))
    oneh = (np.arange(512)[None, :] // 64 == np.arange(8)[:, None]).astype(BF16)

    in_maps = []
    for k in range(NCORE):
        d, s = divmod(k, 4)
        w = wb if d else wf
        in_maps.append({
            "xq": _prep_x(x[s * BS:(s + 1) * BS], reverse=(d == 1)),
            "wih": w["wih"], "whh": w["whh"], "bias": w["bias"],
            "oneh": oneh,
        })
    return in_maps


def kernel(x, W_ih_f, W_hh_f, b_ih_f, b_hh_f, W_ih_b, W_hh_b, b_ih_b, b_hh_b):
    from concourse.bass_utils import run_bass_kernel_spmd

    in_maps = make_in_maps(x, W_ih_f, W_hh_f, b_ih_f, b_hh_f,
                           W_ih_b, W_hh_b, b_ih_b, b_hh_b)

    if "nc" not in _CACHE:
        _CACHE["nc"] = _build_nc()
    nc = _CACHE["nc"]

    res = run_bass_kernel_spmd(nc, in_maps, list(range(NCORE)))

    out = np.empty((B, 2 * H, T), dtype=F32)
    for k in range(NCORE):
        d, s = divmod(k, 4)
        hc = res.results[k]["hout"].reshape(32, 128, 8, 2, 64)
        tmp = hc.transpose(4, 3, 1, 0, 2).reshape(BS, H, T)
        if d == 1:
            tmp = tmp[:, :, ::-1]
        out[s * BS:(s + 1) * BS, d * H:(d + 1) * H, :] = tmp
    return out


# revision 4
# speedup vs baseline: 1.2322x; 1.2322x over previous
import sys

sys.path.insert(0, "/opt/trn_rl_repo")

import numpy as np
import ml_dtypes

BF16 = ml_dtypes.bfloat16

B, C, T = 256, 128, 256
H = 256
BS = 64
NCORE = 8
F32 = np.float32

_CACHE = {}


def _build_nc():
    import concourse.bacc as bacc
    import concourse.tile as tile
    from concourse import mybir

    f32 = mybir.dt.float32
    bf16 = mybir.dt.bfloat16
    Sig = mybir.ActivationFunctionType.Sigmoid
    sub = mybir.AluOpType.subtract
    mult = mybir.AluOpType.mult

    nc = bacc.Bacc("TRN2", target_bir_lowering=False, debug=False,
                   num_devices=NCORE)

    xq_d = nc.dram_tensor("xq", [4, 128, 4096], bf16, kind="ExternalInput").ap()
    wih_d = nc.dram_tensor("wih", [128, 1024], bf16, kind="ExternalInput").ap()
    whh_d = nc.dram_tensor("whh", [128, 2048], bf16, kind="ExternalInput").ap()
    bias_d = nc.dram_tensor("bias", [2, 4, 128], bf16,
                            kind="ExternalInput").ap()
    oneh_d = nc.dram_tensor("oneh", [4, 256], bf16, kind="ExternalInput").ap()
    hout_d = nc.dram_tensor("hout", [32, 128, 1024], f32,
                            kind="ExternalOutput").ap()

    with tile.TileContext(nc) as tc:
        import contextlib
        with contextlib.ExitStack() as ctx:
            const = ctx.enter_context(tc.tile_pool(name="const", bufs=1))
            psum = ctx.enter_context(
                tc.tile_pool(name="psum", bufs=4, space="PSUM"))
            actp = ctx.enter_context(tc.tile_pool(name="actp", bufs=3))
            dvep = ctx.enter_context(tc.tile_pool(name="dvep", bufs=3))
            hrowp = ctx.enter_context(tc.tile_pool(name="hrowp", bufs=2))

            wih_sb = const.tile([128, 1024], bf16)
            whh_sb = const.tile([128, 2048], bf16)
            bias_sb = [const.tile([4, 128], bf16, name=f"bias_sb{h}")
                       for h in range(2)]
            oneh_sb = const.tile([4, 256], bf16)
            nc.sync.dma_start(wih_sb[:], wih_d[:])
            nc.sync.dma_start(whh_sb[:], whh_d[:])
            nc.sync.dma_start(bias_sb[0][:], bias_d[0])
            nc.sync.dma_start(bias_sb[1][:], bias_d[1])
            nc.sync.dma_start(oneh_sb[:], oneh_d[:])
            xq_sb = []
            for q in range(4):
                t_ = const.tile([128, 4096], bf16, name=f"xq_sb{q}")
                nc.sync.dma_start(t_[:], xq_d[q])
                xq_sb.append(t_)

            hbf = [const.tile([128, 128], bf16, name=f"hbf{p}")
                   for p in range(2)]

            psums = {}

            def x_phase(t):
                pa = psum.tile([128, 256], f32, tag="gA", name=f"pa{t}")
                pb = psum.tile([128, 256], f32, tag="gB", name=f"pb{t}")
                psums[t] = (pa, pb)
                q, tl = t // 64, t % 64
                x_rhs = xq_sb[q][:, tl * 64:(tl + 1) * 64]
                for half, ps in ((0, pa), (1, pb)):
                    nc.tensor.matmul(ps[:, :], bias_sb[half][:, :],
                                     oneh_sb[:, :], start=True, stop=False)
                    for mm in range(4):
                        m = half * 4 + mm
                        nc.tensor.matmul(
                            ps[:, mm * 64:(mm + 1) * 64],
                            wih_sb[:, m * 128:(m + 1) * 128],
                            x_rhs,
                            start=False, stop=(t == 0 and mm == 3))

            def h_phase(t):
                pa, pb = psums[t]
                hprev = hbf[(t - 1) % 2]
                for half, ps in ((0, pa), (1, pb)):
                    for mm in range(4):
                        m = half * 4 + mm
                        for hc in range(2):
                            nc.tensor.matmul(
                                ps[:, mm * 64:(mm + 1) * 64],
                                whh_sb[:, hc * 1024 + m * 128:
                                       hc * 1024 + (m + 1) * 128],
                                hprev[:, hc * 64:(hc + 1) * 64],
                                start=False, stop=(mm == 3 and hc == 1))

            c_prev = None
            hrow = None

            def chain(t):
                nonlocal c_prev, hrow
                pa, pb = psums.pop(t)
                tg8 = t % 8
                if tg8 == 0:
                    hrow = hrowp.tile([128, 1024], f32, tag="hrow",
                                      name=f"hrow{t}")
                sig = actp.tile([128, 256], f32, tag="sig", name=f"sig{t}")
                nc.scalar.activation(sig[:], pa[:, :], Sig)
                sf = actp.tile([128, 128], f32, tag="sf", name=f"sf{t}")
                nc.scalar.activation(sf[:], pb[:, 0:128], Sig)
                so = actp.tile([128, 128], f32, tag="so", name=f"so{t}")
                nc.scalar.activation(so[:], pb[:, 128:256], Sig)
                v = dvep.tile([128, 128], f32, tag="v", name=f"v{t}")
                nc.vector.scalar_tensor_tensor(
                    v[:], sig[:, 128:256], 0.5, sig[:, 0:128], sub, mult)
                if t == 0:
                    c_new = v
                else:
                    cf = dvep.tile([128, 128], f32, tag="cf", name=f"cf{t}")
                    nc.vector.tensor_mul(cf[:], sf[:], c_prev[:])
                    c_new = dvep.tile([128, 128], f32, tag="c", name=f"c{t}")
                    nc.vector.tensor_add(c_new[:], cf[:], v[:])
                c_prev = c_new
                sc = actp.tile([128, 128], f32, tag="sc", name=f"sc{t}")
                nc.scalar.activation(sc[:], c_new[:], Sig, scale=4.0)
                nc.vector.scalar_tensor_tensor(
                    hbf[t % 2][:], sc[:], 0.5, so[:], sub, mult)
                nc.vector.scalar_tensor_tensor(
                    hrow[:, tg8 * 128:(tg8 + 1) * 128], sc[:], 0.5, so[:],
                    sub, mult)
                if tg8 == 7:
                    nc.sync.dma_start(hout_d[t // 8], hrow[:])

            x_phase(0)
            x_phase(1)
            for t in range(T):
                if t + 2 < T:
                    x_phase(t + 2)
                if t > 0:
                    h_phase(t)
                chain(t)

    nc.finalize()
    return nc


def _prep_dir(W_ih, W_hh, b_ih, b_hh):
    perm = np.concatenate([np.arange(0, 256), np.arange(512, 768),
                           np.arange(256, 512), np.arange(768, 1024)])
    rs = np.ones(1024, dtype=np.float64)
    rs[256:512] = 2.0
    wih = np.ascontiguousarray((rs[:, None] * W_ih[perm, :]).T).astype(BF16)
    whh_p = 2.0 * rs[:, None] * W_hh[perm, :]
    whh = np.concatenate([whh_p[:, 0:128].T, whh_p[:, 128:256].T],
                         axis=1).astype(BF16)
    bias = (rs * (b_ih + b_hh)[perm]).reshape(2, 4, 128).astype(BF16)
    return {"wih": wih, "whh": whh, "bias": bias}


def _prep_x(x_shard, reverse):
    xt = np.transpose(x_shard, (2, 1, 0))
    if reverse:
        xt = xt[::-1]
    xq = np.ascontiguousarray(
        xt.reshape(4, 64, 128, 64).transpose(0, 2, 1, 3)).reshape(4, 128, 4096)
    return xq.astype(BF16)


def make_in_maps(x, W_ih_f, W_hh_f, b_ih_f, b_hh_f,
                 W_ih_b, W_hh_b, b_ih_b, b_hh_b):
    x = np.asarray(x, dtype=F32)
    wf = _prep_dir(np.asarray(W_ih_f, F32), np.asarray(W_hh_f, F32),
                   np.asarray(b_ih_f, F32), np.asarray(b_hh_f, F32))
    wb = _prep_dir(np.asarray(W_ih_b, F32), np.asarray(W_hh_b, F32),
                   np.asarray(b_ih_b, F32), np.asarray(b_hh_b, F32))
    oneh = (np.arange(256)[None, :] // 64 == np.arange(4)[:, None]).astype(BF16)

    in_maps = []
    for k in range(NCORE):
        d, s = divmod(k, 4)
        w = wb if d else wf
        in_maps.append({
            "xq": _prep_x(x[s * BS:(s + 1) * BS], reverse=(d == 1)),
            "wih": w["wih"], "whh": w["whh"], "bias": w["bias"],
            "oneh": oneh,
        })
    return in_maps


def kernel(x, W_ih_f, W_hh_f, b_ih_f, b_hh_f, W_ih_b, W_hh_b, b_ih_b, b_hh_b):
    from concourse.bass_utils import run_bass_kernel_spmd

    in_maps = make_in_maps(x, W_ih_f, W_hh_f, b_ih_f, b_hh_f,
                           W_ih_b, W_hh_b, b_ih_b, b_hh_b)

    if "nc" not in _CACHE:
        _CACHE["nc"] = _build_nc()
    nc = _CACHE["nc"]

    res = run_bass_kernel_spmd(nc, in_maps, list(range(NCORE)))

    out = np.empty((B, 2 * H, T), dtype=F32)
    for k in range(NCORE):
        d, s = divmod(k, 4)
        hc = res.results[k]["hout"].reshape(32, 128, 8, 2, 64)
        tmp = hc.transpose(4, 3, 1, 0, 2).reshape(BS, H, T)
        if d == 1:
            tmp = tmp[:, :, ::-1]
        out[s * BS:(s + 1) * BS, d * H:(d + 1) * H, :] = tmp
    out *= 2.0
    return out


# revision 8
# speedup vs baseline: 1.3455x; 1.0920x over previous
import sys

sys.path.insert(0, "/opt/trn_rl_repo")

import numpy as np
import ml_dtypes

BF16 = ml_dtypes.bfloat16

B, C, T = 256, 128, 256
H = 256
BS = 64
NCORE = 8
F32 = np.float32

_CACHE = {}


def _build_nc():
    import concourse.bacc as bacc
    import concourse.tile as tile
    from concourse import mybir

    f32 = mybir.dt.float32
    bf16 = mybir.dt.bfloat16
    Sig = mybir.ActivationFunctionType.Sigmoid
    Tanh = mybir.ActivationFunctionType.Tanh
    sub = mybir.AluOpType.subtract
    mult = mybir.AluOpType.mult
    add_op = mybir.AluOpType.add

    nc = bacc.Bacc("TRN2", target_bir_lowering=False, debug=False,
                   num_devices=NCORE)

    xq_d = nc.dram_tensor("xq", [4, 128, 4096], bf16, kind="ExternalInput").ap()
    wih_d = nc.dram_tensor("wih", [128, 1024], bf16, kind="ExternalInput").ap()
    whh_d = nc.dram_tensor("whh", [128, 2048], bf16, kind="ExternalInput").ap()
    bias_d = nc.dram_tensor("bias", [2, 4, 128], bf16,
                            kind="ExternalInput").ap()
    oneh_d = nc.dram_tensor("oneh", [4, 256], bf16, kind="ExternalInput").ap()
    hout_d = nc.dram_tensor("hout", [32, 128, 1024], f32,
                            kind="ExternalOutput").ap()

    with tile.TileContext(nc) as tc:
        import contextlib
        with contextlib.ExitStack() as ctx:
            const = ctx.enter_context(tc.tile_pool(name="const", bufs=1))
            psum = ctx.enter_context(
                tc.tile_pool(name="psum", bufs=4, space="PSUM"))
            actp = ctx.enter_context(tc.tile_pool(name="actp", bufs=3))
            dvep = ctx.enter_context(tc.tile_pool(name="dvep", bufs=3))
            hrowp = ctx.enter_context(tc.tile_pool(name="hrowp", bufs=2))

            wih_sb = const.tile([128, 1024], bf16)
            whh_sb = const.tile([128, 2048], bf16)
            bias_sb = [const.tile([4, 128], bf16, name=f"bias_sb{h}")
                       for h in range(2)]
            oneh_sb = const.tile([4, 256], bf16)
            nc.sync.dma_start(wih_sb[:], wih_d[:])
            nc.sync.dma_start(whh_sb[:], whh_d[:])
            nc.sync.dma_start(bias_sb[0][:], bias_d[0])
            nc.sync.dma_start(bias_sb[1][:], bias_d[1])
            nc.sync.dma_start(oneh_sb[:], oneh_d[:])
            xq_sb = []
            for q in range(4):
                t_ = const.tile([128, 4096], bf16, name=f"xq_sb{q}")
                nc.sync.dma_start(t_[:], xq_d[q])
                xq_sb.append(t_)

            hbf = [const.tile([128, 128], bf16, name=f"hbf{p}")
                   for p in range(2)]

            d0b = [const.tile([128, 512], f32, name=f"d0b{p}")
                   for p in range(2)]
            d1b = [const.tile([128, 512], f32, name=f"d1b{p}")
                   for p in range(2)]
            nc.gpsimd.memset(d0b[0][:], 0.0)
            nc.gpsimd.memset(d0b[1][:], 0.0)
            nc.gpsimd.memset(d1b[0][:], 0.0)
            nc.gpsimd.memset(d1b[1][:], 0.0)

            psums = {}

            def x_phase(t):
                pa = psum.tile([128, 256], f32, tag="gA", name=f"pa{t}")
                pb = psum.tile([128, 256], f32, tag="gB", name=f"pb{t}")
                psums[t] = (pa, pb)
                q, tl = t // 64, t % 64
                x_rhs = xq_sb[q][:, tl * 64:(tl + 1) * 64]
                for half, ps in ((0, pa), (1, pb)):
                    nc.tensor.matmul(ps[:, :], bias_sb[half][:, :],
                                     oneh_sb[:, :], start=True, stop=False)
                    for mm in range(4):
                        m = half * 4 + mm
                        nc.tensor.matmul(
                            ps[:, mm * 64:(mm + 1) * 64],
                            wih_sb[:, m * 128:(m + 1) * 128],
                            x_rhs,
                            start=False, stop=(t == 0 and mm == 3))

            def h_phase(t):
                pa, pb = psums[t]
                hprev = hbf[(t - 1) % 2]
                for half, ps in ((0, pa), (1, pb)):
                    for mm in range(4):
                        m = half * 4 + mm
                        for hc in range(2):
                            nc.tensor.matmul(
                                ps[:, mm * 64:(mm + 1) * 64],
                                whh_sb[:, hc * 1024 + m * 128:
                                       hc * 1024 + (m + 1) * 128],
                                hprev[:, hc * 64:(hc + 1) * 64],
                                start=False, stop=(mm == 3 and hc == 1))

            hrow = None

            def chain(t):
                nonlocal hrow
                pa, pb = psums.pop(t)
                tg8 = t % 8
                if tg8 == 0:
                    hrow = hrowp.tile([128, 1024], f32, tag="hrow",
                                      name=f"hrow{t}")
                p, s = t % 2, t
                sig = actp.tile([128, 256], bf16, tag="sig", name=f"sig{t}")
                nc.scalar.activation(sig[:], pa[:, :], Sig)
                nc.scalar.activation(d0b[p][:, s + 1:s + 257:2],
                                     pb[:, 0:128], Sig)
                so = actp.tile([128, 128], bf16, tag="so", name=f"so{t}")
                nc.scalar.activation(so[:], pb[:, 128:256], Sig)
                nc.vector.scalar_tensor_tensor(
                    d1b[p][:, s + 1:s + 257:2], sig[:, 128:256], 0.5,
                    sig[:, 0:128], sub, mult)
                nc.vector.tensor_tensor_scan(
                    d1b[1 - p][:, s:s + 256], d0b[p][:, s:s + 256],
                    d1b[p][:, s:s + 256], 0.0, mult, add_op)
                tc_t = actp.tile([128, 128], bf16, tag="tc", name=f"tc{t}")
                nc.scalar.activation(tc_t[:], d1b[1 - p][:, s + 1:s + 257:2],
                                     Tanh, scale=2.0)
                nc.vector.tensor_mul(hbf[t % 2][:], tc_t[:], so[:])
                nc.vector.tensor_mul(
                    hrow[:, tg8 * 128:(tg8 + 1) * 128], tc_t[:], so[:])
                if tg8 == 7:
                    nc.sync.dma_start(hout_d[t // 8], hrow[:])

            x_phase(0)
            x_phase(1)
            for t in range(T):
                if t + 2 < T:
                    x_phase(t + 2)
                if t > 0:
                    h_phase(t)
                chain(t)

    nc.finalize()
    return nc


def _prep_dir(W_ih, W_hh, b_ih, b_hh):
    perm = np.concatenate([np.arange(0, 256), np.arange(512, 768),
                           np.arange(256, 512), np.arange(768, 1024)])
    rs = np.ones(1024, dtype=np.float64)
    rs[256:512] = 2.0
    wih = np.ascontiguousarray((rs[:, None] * W_ih[perm, :]).T).astype(BF16)
    whh_p = rs[:, None] * W_hh[perm, :]
    whh = np.concatenate([whh_p[:, 0:128].T, whh_p[:, 128:256].T],
                         axis=1).astype(BF16)
    bias = (rs * (b_ih + b_hh)[perm]).reshape(2, 4, 128).astype(BF16)
    return {"wih": wih, "whh": whh, "bias": bias}


def _prep_x(x_shard, reverse):
    xt = np.transpose(x_shard, (2, 1, 0))
    if reverse:
        xt = xt[::-1]
    xq = np.ascontiguousarray(
        xt.reshape(4, 64, 128, 64).transpose(0, 2, 1, 3)).reshape(4, 128, 4096)
    return xq.astype(BF16)


def make_in_maps(x, W_ih_f, W_hh_f, b_ih_f, b_hh_f,
                 W_ih_b, W_hh_b, b_ih_b, b_hh_b):
    x = np.asarray(x, dtype=F32)
    wf = _prep_dir(np.asarray(W_ih_f, F32), np.asarray(W_hh_f, F32),
                   np.asarray(b_ih_f, F32), np.asarray(b_hh_f, F32))
    wb = _prep_dir(np.asarray(W_ih_b, F32), np.asarray(W_hh_b, F32),
                   np.asarray(b_ih_b, F32), np.asarray(b_hh_b, F32))
    oneh = (np.arange(256)[None, :] // 64 == np.arange(4)[:, None]).astype(BF16)

    in_maps = []
    for k in range(NCORE):
        d, s = divmod(k, 4)
        w = wb if d else wf
        in_maps.append({
            "xq": _prep_x(x[s * BS:(s + 1) * BS], reverse=(d == 1)),
            "wih": w["wih"], "whh": w["whh"], "bias": w["bias"],
            "oneh": oneh,
        })
    return in_maps


def kernel(x, W_ih_f, W_hh_f, b_ih_f, b_hh_f, W_ih_b, W_hh_b, b_ih_b, b_hh_b):
    from concourse.bass_utils import run_bass_kernel_spmd

    in_maps = make_in_maps(x, W_ih_f, W_hh_f, b_ih_f, b_hh_f,
                           W_ih_b, W_hh_b, b_ih_b, b_hh_b)

    if "nc" not in _CACHE:
        _CACHE["nc"] = _build_nc()
    nc = _CACHE["nc"]

    res = run_bass_kernel_spmd(nc, in_maps, list(range(NCORE)))

    out = np.empty((B, 2 * H, T), dtype=F32)
    for k in range(NCORE):
        d, s = divmod(k, 4)
        hc = res.results[k]["hout"].reshape(32, 128, 8, 2, 64)
        tmp = hc.transpose(4, 3, 1, 0, 2).reshape(BS, H, T)
        if d == 1:
            tmp = tmp[:, :, ::-1]
        out[s * BS:(s + 1) * BS, d * H:(d + 1) * H, :] = tmp
    return out
